# revision 35
# baseline (speedup 1.0000x reference)
"""Trainium2 Bass kernel for nn_Cross_Frequency_Enhanced_Block.

kernel(**inputs) takes FULL unsharded inputs (as in setup_inputs()) and
returns the FULL (32, 1024, 512) float32 output.

Sharding: data-parallel over batch B across 8 NeuronCores (4 batches/core);
the 32 Fourier modes of the complex-weight einsum are mode-sharded (4/core)
around a pair of AllToAlls.

Algorithm notes (validated vs reference in numpy, absmax ~4e-6 in f32;
~3e-3 rel with the bf16 layout below):
  - rfft(x @ Wq.T)[:, :32] == Wq @ rfft(x)[:, :32]: DFT x once per batch via
    matmuls against cos/sin tables (only 32 modes needed), apply Wq/Wk in the
    frequency domain; q/k never materialize in the time domain.
  - complex tanh via the stable sech formula with Cody-Waite range reduction
    for sin/cos (ACT Sin domain is [-pi, pi]).  The q/k/Z/tanh chain stays
    f32 (the trig phase is sensitive); everything downstream is bf16.
  - per-mode complex weight einsum: stationary [VR|VI] / [-VI|VR] column
    pairs, moving = w mode-slab (bf16, N=512).
  - irfft as matmul against a (64, 1024) table (1/(D*D) and 2/L folded in).
  - moving average (k=128, edge replicate) via prefix scan + shifted
    differences (f32).  u - mov(u) kills any constant bias, so bo drops.
  - BatchNorm(eval) folded into the final PE-transpose eviction as per-l
    scale/bias on ACT.

Schedule: x is loaded once per batch (bf16) and both consumed by the DFT
and transposed for stage E immediately; stage-E tiles are double-buffered
so the 4 batches pipeline; PSUM evictions are spread across DVE/ACT/Pool.
"""

import os
from contextlib import ExitStack

import numpy as np
import ml_dtypes

import concourse.bacc as bacc
import concourse.bass as bass
import concourse.tile as tile
import concourse.mybir as mybir
from concourse.bass_utils import run_bass_kernel_spmd

B, L, D, MODES = 32, 1024, 512, 32
NCORES = 8
BPC = B // NCORES
F32 = mybir.dt.float32
FR = mybir.dt.float32r
BF = mybir.dt.bfloat16
AF = mybir.ActivationFunctionType
ALU = mybir.AluOpType

MAGIC = float(np.float32(12582912.0))        # 1.5*2^23 round-to-nearest
CW1 = float(np.float32(6.28125))             # 2pi hi (exact in f32)
CW2 = float(2 * np.pi - 6.28125)             # 2pi lo
INV2PI = float(np.float32(1.0 / (2 * np.pi)))
PI = float(np.float32(np.pi))
SIM_GELU = int(os.environ.get("BK_SIM_GELU", "0"))


def _tables():
    l_ = np.arange(L)[:, None].astype(np.float64)
    m_ = np.arange(MODES)[None, :].astype(np.float64)
    ang = 2 * np.pi * l_ * m_ / L
    F = np.concatenate([np.cos(ang), -np.sin(ang)], 1).astype(np.float32)
    ftab = np.ascontiguousarray(F.reshape(8, 128, 64).transpose(1, 0, 2))

    a = np.full((MODES,), 2.0 / L)
    a[0] = 1.0 / L
    a = a / (D * D)
    Gc = a[:, None] * np.cos(2 * np.pi * m_.T * l_.T / L)
    Gs = a[:, None] * -np.sin(2 * np.pi * m_.T * l_.T / L)
    gtab = np.concatenate([Gc, Gs], 0).astype(np.float32)

    ident = np.eye(128, dtype=np.float32)

    # banded-matrix form of the k=128 edge-replicated moving average:
    # mov = A^T u with A[j,l] block-tridiagonal; only 5 distinct 128x128
    # blocks (all entries k/128 -> exact in bf16)
    A = np.zeros((L, L), np.float64)
    for l in range(L):
        lo, hi = max(0, l - 64), min(L - 1, l + 63)
        A[lo:hi + 1, l] += 1 / 128
        if l < 64:
            A[0, l] += (64 - l) / 128
        if l > 960:
            A[L - 1, l] += (l - 960) / 128
    K = 128
    blk = lambda a, b: A[a * K:(a + 1) * K, b * K:(b + 1) * K]
    mtab = np.ascontiguousarray(np.stack(
        [blk(3, 3), blk(2, 3), blk(4, 3), blk(0, 0), blk(7, 7)],
        axis=1).astype(np.float32))           # [128, 5, 128]
    return ftab, gtab, ident, mtab


def _t128(w):
    """(512, 512) host array -> (128, 4, 512) [p, ch, col] with row=ch*128+p."""
    return np.ascontiguousarray(w.reshape(4, 128, 512).transpose(1, 0, 2))


def _build():
    nc = bacc.Bacc("TRN2", target_bir_lowering=False, debug=False,
                   num_devices=NCORES)
    dram = {}

    def din(name, shape, dt=FR):
        dram[name] = nc.dram_tensor(name, list(shape), dt,
                                    kind="ExternalInput").ap()

    MLOC = MODES // NCORES               # modes owned per core
    din("xs", (BPC, L, D), BF)
    din("wslab", (MLOC, 2, D, D), BF)    # per-core mode slice of w
    din("wqt", (128, 4, D), FR)
    din("wkt", (128, 4, D), FR)
    for n in ("wot", "w1t", "w2t"):
        din(n, (128, 4, D), BF)
    din("ftab", (128, 8, 64), BF)
    din("gtab", (64, L), BF)
    din("identt", (128, 128), FR)
    din("identtb", (128, 128), BF)
    din("mtab", (128, 5, 128), BF)
    din("bnt", (128, 8, 4), F32)
    din("bqkt", (128, 4, 2), F32)
    out_d = nc.dram_tensor("out", [BPC, L, D], F32, kind="ExternalOutput").ap()
    # collective staging: xqkv -> mode owners, einsum result -> batch owners
    vq_d = nc.dram_tensor("vq_d", [BPC, 64, D], BF).ap()
    vq_snd = nc.dram_tensor("vq_snd", [NCORES, BPC, 2, MLOC, D], BF).ap()
    vq_rcv = nc.dram_tensor("vq_rcv", [NCORES, BPC, 2, MLOC, D], BF).ap()
    xwm_d = nc.dram_tensor("xwm_d", [MLOC, 64, D], BF).ap()
    xw_snd = nc.dram_tensor("xw_snd", [NCORES, MLOC, 2, BPC, D], BF).ap()
    xw_rcv = nc.dram_tensor("xw_rcv", [NCORES, MLOC, 2, BPC, D], BF).ap()

    with tile.TileContext(nc) as tc, ExitStack() as ctx:
        con = ctx.enter_context(tc.tile_pool(name="con", bufs=1))
        xtp = ctx.enter_context(tc.tile_pool(name="xtp", bufs=1))
        outp = ctx.enter_context(tc.tile_pool(name="outp", bufs=2))
        ps = ctx.enter_context(tc.tile_pool(name="ps", bufs=4, space="PSUM"))
        ps2 = ctx.enter_context(tc.tile_pool(name="ps2", bufs=2, space="PSUM"))
        # stages A-D scratch lives in `wrk`, released before stage E's `ep`
        # is allocated so the two share the same SBUF region (LIFO reuse)
        wrk = tc.alloc_tile_pool(name="wrk", bufs=1)

        def cload(name, shape, dt=FR, eng=None):
            t = con.tile(list(shape), dt, tag=name)
            (eng or nc.sync).dma_start(out=t[:], in_=dram[name])
            return t

        # spread startup loads over queues: sync gets only what stage A
        # needs first; the big weights arrive on other queues in background
        ftab = cload("ftab", (128, 8, 64), BF)
        ident = cload("identt", (128, 128))
        identb = cload("identtb", (128, 128), BF)
        gtab = cload("gtab", (64, L), BF, nc.scalar)
        wqt = cload("wqt", (128, 4, D), FR, nc.gpsimd)
        wkt = cload("wkt", (128, 4, D), FR, nc.gpsimd)
        wot = cload("wot", (128, 4, D), BF, nc.scalar)
        w1t = cload("w1t", (128, 4, D), BF, nc.scalar)
        w2t = cload("w2t", (128, 4, D), BF, nc.scalar)
        mtab = cload("mtab", (128, 5, 128), BF, nc.scalar)
        bnt = cload("bnt", (128, 8, 4), F32, nc.scalar)
        bqkt = cload("bqkt", (128, 4, 2), F32, nc.scalar)
        # whole per-core mode-slab of w (2 MB bf16) preloaded once; stage D
        # then runs from SBUF instead of racing per-tile DMAs
        wsl = con.tile([128, 4, 2, 4, 512], BF, tag="wsl")
        nc.gpsimd.dma_start(
            out=wsl[:],
            in_=dram["wslab"].rearrange("m r (c p) d -> p m r c d", p=128))

        def tt(o, a, bb, op):
            nc.vector.tensor_tensor(o, a, bb, op)

        # --- BN constants: c1 = gamma*rsqrt(var+eps), c0 = beta - mean*c1
        c1l = con.tile([128, 8], F32)
        c0l = con.tile([128, 8], F32)
        sq = con.tile([128, 8], F32)
        ve = con.tile([128, 8], F32)
        yy = con.tile([128, 8], F32)
        nc.vector.tensor_scalar_add(ve[:], bnt[:, :, 3], 1e-5)
        nc.scalar.activation(sq[:], ve[:], AF.Sqrt)
        nc.vector.reciprocal(c1l[:], sq[:])
        tt(yy[:], c1l[:], c1l[:], ALU.mult)
        tt(yy[:], yy[:], ve[:], ALU.mult)
        nc.vector.tensor_scalar(yy[:], yy[:], -0.5, 1.5, ALU.mult, ALU.add)
        tt(c1l[:], c1l[:], yy[:], ALU.mult)
        tt(c1l[:], c1l[:], bnt[:, :, 0], ALU.mult)
        tt(c0l[:], bnt[:, :, 2], c1l[:], ALU.mult)
        tt(c0l[:], bnt[:, :, 1], c0l[:], ALU.subtract)
        bqs = con.tile([128, 4, 2], FR)
        nc.scalar.activation(bqs[:], bqkt[:], AF.Copy, scale=float(L))

        # ============ stage A: x load (kept resident) + 32-mode DFT ========
        xf_all = wrk.tile([128, 4, BPC, 64], FR, tag="xfa")
        xts = [xtp.tile([128, 8, D], BF, tag=f"xt{b}", name=f"xt{b}")
               for b in range(BPC)]
        for b in range(BPC):
            xt = xts[b]
            nc.sync.dma_start(
                out=xt[:],
                in_=dram["xs"][b].rearrange("(t p) d -> p t d", p=128))
            xfT_ps = ps.tile([128, 512], F32, tag="ps")
            for lt in range(8):
                nc.tensor.matmul(xfT_ps[0:64, :], ftab[:, lt, :],
                                 xt[:, lt, :],
                                 start=(lt == 0), stop=(lt == 7))
            xfT_sb = wrk.tile([64, 512], FR, tag="xfT", bufs=2)
            nc.vector.tensor_copy(xfT_sb[:], xfT_ps[0:64, :])
            xf_ps = ps.tile([128, 512], FR, tag="ps")
            for dch in range(4):
                nc.tensor.transpose(xf_ps[:, dch * 64:dch * 64 + 64],
                                    xfT_sb[:, dch * 128:dch * 128 + 128],
                                    ident[0:64, 0:64])
            nc.vector.tensor_copy(
                xf_all[:, :, b, :],
                xf_ps[:, 0:256].rearrange("p (c m) -> p c m", c=4))

        # =================== stage B: qf/kf in frequency domain ============
        qkf = wrk.tile([128, 4, BPC, 128], FR, tag="qkf")
        for wt, co in ((wqt, 0), (wkt, 64)):
            for ech in range(4):
                qp = ps.tile([128, 512], F32, tag="ps")
                for dch in range(4):
                    nc.tensor.matmul(
                        qp[:, 0:256], wt[:, dch, ech * 128:ech * 128 + 128],
                        xf_all[:, dch, :, :],
                        start=(dch == 0), stop=(dch == 3))
                nc.vector.tensor_copy(
                    qkf[:, ech, :, co:co + 64],
                    qp[:, 0:256].rearrange("p (b m) -> p b m", b=BPC))
        # bias: mode-0 real += L*b  (DFT of constant vector)
        for ech in range(4):
            for co, j in ((0, 0), (64, 1)):
                tt(qkf[:, ech, :, co:co + 1], qkf[:, ech, :, co:co + 1],
                   bqs[:, ech:ech + 1, j:j + 1].to_broadcast([128, BPC, 1]),
                   ALU.add)

        # =================== stage C: Z, tanh, U, xqkv =====================
        ZpsR = ps.tile([32, 512], F32, tag="ps")
        ZpsI = ps.tile([32, 512], F32, tag="ps")
        for b in range(BPC):
            for ech in range(4):
                nc.tensor.matmul(
                    ZpsR[0:32, b * 64:b * 64 + 64],
                    qkf[:, ech, b, 0:32], qkf[:, ech, b, 64:128],
                    start=(ech == 0), stop=(ech == 3))
                nc.tensor.matmul(
                    ZpsI[0:32, b * 64:b * 64 + 64],
                    qkf[:, ech, b, 32:64], qkf[:, ech, b, 64:128],
                    start=(ech == 0), stop=(ech == 3))
        ZsbR = wrk.tile([32, BPC, 64], F32)
        ZsbI = wrk.tile([32, BPC, 64], F32)
        nc.vector.tensor_copy(
            ZsbR[:], ZpsR[0:32, 0:256].rearrange("p (b y) -> p b y", b=BPC))
        nc.vector.tensor_copy(
            ZsbI[:], ZpsI[0:32, 0:256].rearrange("p (b y) -> p b y", b=BPC))

        sh = [32, BPC, 32]
        zr = wrk.tile(sh, F32)
        zi = wrk.tile(sh, F32)
        # Z = (QR + iQI).(KR + iKI):  Re = QR.KR - QI.KI, Im = QR.KI + QI.KR
        tt(zr[:], ZsbR[:, :, 0:32], ZsbI[:, :, 32:64], ALU.subtract)
        tt(zi[:], ZsbR[:, :, 32:64], ZsbI[:, :, 0:32], ALU.add)
        tht = wrk.tile(sh, F32)
        sech = wrk.tile(sh, F32)
        s2y = wrk.tile(sh, F32)
        c2y = wrk.tile(sh, F32)
        w1 = wrk.tile(sh, F32)
        w2 = wrk.tile(sh, F32)
        w3 = wrk.tile(sh, F32)
        nc.scalar.activation(tht[:], zr[:], AF.Tanh, scale=2.0)
        nc.scalar.activation(w1[:], zr[:], AF.Abs, scale=2.0)
        nc.vector.tensor_scalar_min(w1[:], w1[:], 87.0)
        nc.scalar.activation(w1[:], w1[:], AF.Exp, scale=-1.0)   # e^-2|x|
        tt(w2[:], w1[:], w1[:], ALU.mult)
        nc.vector.tensor_scalar_add(w2[:], w2[:], 1.0)
        nc.vector.reciprocal(w2[:], w2[:])
        tt(sech[:], w1[:], w2[:], ALU.mult)
        nc.vector.tensor_scalar(sech[:], sech[:], 2.0, None, ALU.mult)
        for dst, ofs in ((s2y, 0.0), (c2y, 0.25)):
            # k = round(2*zi/(2pi) + ofs) via the 1.5*2^23 magic-add trick
            nc.vector.tensor_scalar(w1[:], zi[:], 2.0 * INV2PI, MAGIC + ofs,
                                    ALU.mult, ALU.add)
            nc.vector.tensor_scalar_sub(w1[:], w1[:], MAGIC)
            # red = 2*zi (+ pi/2 for cos) - k*CW1 - k*CW2, clamp to [-pi, pi]
            nc.vector.tensor_scalar(w2[:], zi[:], 2.0, ofs * 2.0 * PI,
                                    ALU.mult, ALU.add)
            nc.vector.tensor_scalar(w3[:], w1[:], CW1, None, ALU.mult)
            tt(w2[:], w2[:], w3[:], ALU.subtract)
            nc.vector.tensor_scalar(w3[:], w1[:], CW2, None, ALU.mult)
            tt(w2[:], w2[:], w3[:], ALU.subtract)
            nc.vector.tensor_scalar(w2[:], w2[:], -PI, PI, ALU.max, ALU.min)
            nc.scalar.activation(dst[:], w2[:], AF.Sin)
        tt(w1[:], c2y[:], sech[:], ALU.mult)
        nc.vector.tensor_scalar_add(w1[:], w1[:], 1.0)
        nc.vector.reciprocal(w1[:], w1[:])                       # 1/den
        TR = wrk.tile(sh, FR)
        TI = wrk.tile(sh, FR)
        tt(TR[:], tht[:], w1[:], ALU.mult)
        tt(TI[:], s2y[:], sech[:], ALU.mult)
        tt(TI[:], TI[:], w1[:], ALU.mult)
        # U1 = [TR^T | TI^T], U2 = [-TI^T | TR^T] per batch (start part 0)
        U1 = wrk.tile([32, BPC, 64], BF)
        U2 = wrk.tile([32, BPC, 64], BF)
        for b in range(BPC):
            tp1 = ps.tile([32, 128], FR, tag="ps")
            nc.tensor.transpose(tp1[0:32, 0:32], TR[:, b, :],
                                ident[0:32, 0:32])
            nc.tensor.transpose(tp1[0:32, 64:96], TI[:, b, :],
                                ident[0:32, 0:32])
            nc.vector.tensor_copy(U1[:, b, 0:32], tp1[0:32, 0:32])
            nc.vector.tensor_copy(U1[:, b, 32:64], tp1[0:32, 64:96])
            nc.scalar.activation(U2[:, b, 0:32], tp1[0:32, 64:96],
                                 AF.Copy, scale=-1.0)
            nc.vector.tensor_copy(U2[:, b, 32:64], tp1[0:32, 0:32])
        # xqkv per b -> transpose to (col, e) and stage to DRAM for AllToAll
        for b in range(BPC):
            kpsR = ps.tile([32, 512], FR, tag="ps")
            kpsI = ps.tile([32, 512], FR, tag="ps")
            for ech in range(4):
                nc.tensor.transpose(kpsR[0:32, ech * 128:ech * 128 + 128],
                                    qkf[:, ech, b, 64:96], ident[:])
                nc.tensor.transpose(kpsI[0:32, ech * 128:ech * 128 + 128],
                                    qkf[:, ech, b, 96:128], ident[:])
            kfTR = wrk.tile([32, 512], BF, tag="kfTR", bufs=2)
            kfTI = wrk.tile([32, 512], BF, tag="kfTI", bufs=2)
            nc.vector.tensor_copy(kfTR[:], kpsR[0:32, :])
            nc.scalar.copy(kfTI[:], kpsI[0:32, :])
            vps = ps.tile([128, 512], F32, tag="ps")
            for ech in range(4):
                nc.tensor.matmul(vps[:, ech * 64:ech * 64 + 64],
                                 kfTR[:, ech * 128:ech * 128 + 128],
                                 U1[:, b, :], start=True, stop=False)
                nc.tensor.matmul(vps[:, ech * 64:ech * 64 + 64],
                                 kfTI[:, ech * 128:ech * 128 + 128],
                                 U2[:, b, :], start=False, stop=True)
            vsb = wrk.tile([128, 4, 64], BF, tag="vsb", bufs=2)
            nc.vector.tensor_copy(
                vsb[:], vps[:, 0:256].rearrange("p (c m) -> p c m", c=4))
            vTp = ps.tile([64, 512], BF, tag="psb", bufs=2)
            for ech in range(4):
                nc.tensor.transpose(vTp[0:64, ech * 128:ech * 128 + 128],
                                    vsb[:, ech, :], identb[:])
            vT_sb = wrk.tile([64, 512], BF, tag="vT", bufs=2)
            nc.scalar.copy(vT_sb[:], vTp[0:64, :])
            nc.sync.dma_start(out=vq_d[b], in_=vT_sb[:])

        # =================== stage D: AllToAll + mode-sharded einsum =======
        # exchange 1: route each core's xqkv columns to the mode owner
        grp = [list(range(NCORES))]
        nc.sync.dma_start(
            out=vq_snd[:],
            in_=vq_d.rearrange("b (k j m) e -> j b k m e", k=2, j=NCORES,
                               m=MLOC))
        nc.gpsimd.collective_compute(
            "AllToAll", ALU.bypass, replica_groups=grp,
            ins=[vq_snd.opt()], outs=[vq_rcv.opt()])
        # xqgT: partition p = kind*32 + b_global, free (m_local, e)
        xqgT = wrk.tile([64, MLOC, D], BF, tag="xqgT")
        for k in range(2):
            nc.sync.dma_start(
                out=xqgT[k * 32:k * 32 + 32, :, :],
                in_=vq_rcv[:, :, k, :, :].rearrange("s b m e -> (s b) m e"))
        # transpose back to (e-part, cols=(kind, b_global)) per (m', ech)
        xqa = wrk.tile([128, 4, MLOC, 64], BF, tag="xqa")
        xqa2 = wrk.tile([128, 4, MLOC, 64], BF, tag="xqa2")
        for ml in range(MLOC):
            xp = ps.tile([128, 512], BF, tag="psb", bufs=2)
            for ech in range(4):
                nc.tensor.transpose(xp[:, ech * 64:ech * 64 + 64],
                                    xqgT[0:64, ml, ech * 128:ech * 128 + 128],
                                    identb[0:64, 0:64])
            nc.vector.tensor_copy(
                xqa[:, :, ml, :],
                xp[:, 0:256].rearrange("p (c m) -> p c m", c=4))
        nc.scalar.activation(xqa2[:, :, :, 0:32], xqa[:, :, :, 32:64],
                             AF.Copy, scale=-1.0)
        nc.gpsimd.tensor_copy(xqa2[:, :, :, 32:64], xqa[:, :, :, 0:32])
        _nmodes = 0 if os.environ.get("BK_SKIP_D") else MLOC
        for ml in range(_nmodes):
            pm = ps2.tile([64, 512], F32, tag="ps2")
            for ech in range(4):
                nc.tensor.matmul(pm[:], xqa[:, ech, ml, :],
                                 wsl[:, ml, 0, ech, :],
                                 start=(ech == 0), stop=False)
                nc.tensor.matmul(pm[:], xqa2[:, ech, ml, :],
                                 wsl[:, ml, 1, ech, :],
                                 start=False, stop=(ech == 3))
            xw_sb = outp.tile([64, 512], BF, tag="mid3")
            nc.vector.tensor_copy(xw_sb[:], pm[:])
            nc.sync.dma_start(out=xwm_d[ml], in_=xw_sb[:])
        # exchange 2: route per-mode results back to batch owners
        nc.sync.dma_start(
            out=xw_snd[:],
            in_=xwm_d.rearrange("m (r j bl) o -> j m r bl o", r=2, j=NCORES,
                                bl=BPC))
        nc.gpsimd.collective_compute(
            "AllToAll", ALU.bypass, replica_groups=grp,
            ins=[xw_snd.opt()], outs=[xw_rcv.opt()])

        # =================== stage E: irfft, Wo, MA, convs, BN =============
        wrk.release()
        ep = tc.alloc_tile_pool(name="ep", bufs=1)
        ctx.callback(ep.release)

        # banded moving-average block schedule: (mtab idx, source l-chunk)
        MOVB = {0: [(3, 0), (2, 1)], 7: [(1, 6), (4, 7)]}
        for _c in range(1, 7):
            MOVB[_c] = [(1, _c - 1), (0, _c), (2, _c + 1)]

        def e_head(b):
            """XXT load, irfft, u^T = fre^T Wo + x^T, banded mov, xd."""
            XXT = ep.tile([64, 512], BF, tag="xxt", bufs=2, name="XXT")
            for r in range(2):
                nc.sync.dma_start(
                    out=XXT[r * 32:r * 32 + 32, :],
                    in_=xw_rcv[:, :, r, b, :].rearrange("s m o -> (s m) o"))
            fre = ep.tile([128, 4, L], BF, tag="fre", bufs=2, name="fre")
            for och in range(4):
                for lh in range(2):
                    fp = ps2.tile([128, 512], F32, tag="ps2", name="fp")
                    nc.tensor.matmul(fp[:],
                                     XXT[:, och * 128:och * 128 + 128],
                                     gtab[:, lh * 512:lh * 512 + 512],
                                     start=True, stop=True)
                    dst = fre[:, och, lh * 512:lh * 512 + 512]
                    if (och + lh) % 2 == 0:
                        nc.vector.tensor_copy(dst, fp[:])
                    else:
                        nc.scalar.copy(dst, fp[:])
            # u^T (l-part, d) = fre^T Wo + x^T: fre slices stationary, wot
            # moving; the residual rides in as identity @ xt (already l-part)
            uT = ep.tile([128, 8, D], BF, tag="uT", bufs=2, name="uT")
            for lc in range(8):
                up = ps.tile([128, 512], F32, tag="ps", name="up")
                for och in range(4):
                    nc.tensor.matmul(
                        up[:], fre[:, och, lc * 128:lc * 128 + 128],
                        wot[:, och, :], start=(och == 0), stop=(och == 3))
                # residual: u^T = Wo-psum + x^T (xt is already l-partitioned)
                tt(uT[:, lc, :], up[:], xts[b][:, lc, :], ALU.add)
            # mov via banded matmul; xd^T = u^T - mov^T, then transpose back
            xdT = ep.tile([128, 8, D], BF, tag="xdT", bufs=2, name="xdT")
            for lc in range(8):
                mp = ps2.tile([128, 512], F32, tag="ps2", name="mp")
                for i, (mi, src) in enumerate(MOVB[lc]):
                    nc.tensor.matmul(mp[:], mtab[:, mi, :], uT[:, src, :],
                                     start=(i == 0),
                                     stop=(i == len(MOVB[lc]) - 1))
                tt(xdT[:, lc, :], uT[:, lc, :], mp[:], ALU.subtract)
            xd = ep.tile([128, 4, L], BF, tag="xd", bufs=2, name="xd")
            for dch in range(4):
                for lh in range(2):
                    tp0 = ps.tile([128, 512], BF, tag="psb", bufs=2,
                                  name="tp0")
                    for lq in range(4):
                        lc = lh * 4 + lq
                        nc.tensor.transpose(
                            tp0[:, lq * 128:lq * 128 + 128],
                            xdT[:, lc, dch * 128:dch * 128 + 128], identb[:])
                    dst = xd[:, dch, lh * 512:lh * 512 + 512]
                    if (dch + lh) % 2 == 0:
                        nc.vector.tensor_copy(dst, tp0[:])
                    else:
                        nc.scalar.copy(dst, tp0[:])
            return xd, xdT

        def e_tail(b, xd, xdT):
            """conv1+gelu, conv2+residual, transpose+BN, store."""
            y1g = ep.tile([128, 4, L], BF, tag="y1g", name="y1g")
            for och in range(4):
                for lh in range(2):
                    cp = ps2.tile([128, 512], F32, tag="ps2", name="cp")
                    for dch in range(4):
                        nc.tensor.matmul(
                            cp[:], w1t[:, dch, och * 128:och * 128 + 128],
                            xd[:, dch, lh * 512:lh * 512 + 512],
                            start=(dch == 0), stop=(dch == 3))
                    yslc = y1g[:, och, lh * 512:lh * 512 + 512]
                    if not SIM_GELU:
                        nc.scalar.activation(yslc, cp[:], AF.Gelu)
                    elif SIM_GELU == 2:
                        # timing-only stand-in (same ACT cost, wrong values)
                        nc.scalar.activation(yslc, cp[:], AF.Tanh)
                    else:
                        # CoreSim has no Gelu LUT: tanh-approx stand-in
                        y1c = ep.tile([128, 512], F32, tag="gel1", name="y1c")
                        nc.scalar.copy(y1c[:], cp[:])
                        sqt = ep.tile([128, 512], F32, tag="gel2", name="sqt")
                        nc.scalar.activation(sqt[:], y1c[:], AF.Square)
                        tt(sqt[:], sqt[:], y1c[:], ALU.mult)
                        nc.vector.tensor_scalar(sqt[:], sqt[:], 0.044715,
                                                None, ALU.mult)
                        tt(sqt[:], sqt[:], y1c[:], ALU.add)
                        nc.vector.tensor_scalar(sqt[:], sqt[:],
                                                0.7978845608028654,
                                                None, ALU.mult)
                        nc.scalar.activation(sqt[:], sqt[:], AF.Tanh)
                        nc.vector.tensor_scalar(sqt[:], sqt[:], 0.5, 0.5,
                                                ALU.mult, ALU.add)
                        tt(yslc, y1c[:], sqt[:], ALU.mult)
            # conv2 in transposed form: res^T (l-part, d) = y1g^T W2 + xd^T,
            # so BN's per-l scale/bias applies per-partition and the final
            # PE transpose stage disappears
            for lc in range(8):
                rp = ps.tile([128, 512], F32, tag="ps", name="rp")
                for och in range(4):
                    nc.tensor.matmul(
                        rp[:], y1g[:, och, lc * 128:lc * 128 + 128],
                        w2t[:, och, :], start=(och == 0), stop=(och == 3))
                rsb = ep.tile([128, 512], FR, tag="rsb", bufs=2, name="rsb")
                tt(rsb[:], rp[:], xdT[:, lc, :], ALU.add)
                ob = outp.tile([128, 512], F32, tag="ob", name="ob")
                nc.scalar.activation(ob[:], rsb[:], AF.Identity,
                                     bias=c0l[:, lc:lc + 1],
                                     scale=c1l[:, lc:lc + 1])
                nc.sync.dma_start(out=out_d[b, lc * 128:lc * 128 + 128, :],
                                  in_=ob[:])

        # software pipeline: batch b+1's head runs while batch b's convs
        # occupy the PE; engine queues then order head work first
        _bpce = 0 if os.environ.get("BK_SKIP_E") else BPC
        xds = {}
        sched = []
        for b in range(_bpce):
            sched.append(("h", b))
            if b >= 1:
                sched.append(("t", b - 1))
        if _bpce:
            sched.append(("t", _bpce - 1))
        for kind, b in sched:
            if kind == "h":
                xds[b] = e_head(b)
            else:
                e_tail(b, *xds.pop(b))

    nc.compile()
    return nc


_CACHE = {}


def _get_nc():
    if "nc" not in _CACHE:
        _CACHE["nc"] = _build()
    return _CACHE["nc"]


def _fingerprint(inputs):
    import zlib
    h = 0
    for k in sorted(inputs):
        v = np.asarray(inputs[k])
        h = zlib.adler32(str((k, v.shape, str(v.dtype))).encode(), h)
        raw = v.reshape(-1)
        step = max(1, raw.size // 1024)
        h = zlib.adler32(np.ascontiguousarray(raw[::step][:1024]).tobytes(), h)
        h = zlib.adler32(np.ascontiguousarray(raw[-3:]).tobytes(), h)
    return h


def _host_inputs(inputs):
    fp = _fingerprint(inputs)
    if _CACHE.get("fp") == fp:
        return _CACHE["in_maps"]
    bf16 = ml_dtypes.bfloat16
    x = np.asarray(inputs["x"], dtype=np.float32)
    ftab, gtab, ident, mtab = _tables()
    wr = np.asarray(inputs["w_real"], dtype=np.float32)[0]   # (E, O, MODES)
    wi = np.asarray(inputs["w_imag"], dtype=np.float32)[0]
    wslab = np.ascontiguousarray(
        np.stack([wr.transpose(2, 0, 1), wi.transpose(2, 0, 1)],
                 axis=1).astype(bf16))
    bn = [np.asarray(inputs[k], dtype=np.float32)
          for k in ("bn_gamma", "bn_beta", "bn_mean", "bn_var")]
    bnt = np.ascontiguousarray(
        np.stack(bn, -1).reshape(8, 128, 4).transpose(1, 0, 2))
    bq = np.asarray(inputs["bq"], dtype=np.float32)
    bk = np.asarray(inputs["bk"], dtype=np.float32)
    bqkt = np.ascontiguousarray(
        np.stack([bq.reshape(4, 128).T, bk.reshape(4, 128).T], -1))
    com = {
        "wqt": _t128(np.asarray(inputs["Wq"], np.float32).T),
        "wkt": _t128(np.asarray(inputs["Wk"], np.float32).T),
        "wot": _t128(np.asarray(inputs["Wo"], np.float32).T).astype(bf16),
        "w1t": _t128(np.asarray(inputs["conv1_w"], np.float32).T).astype(bf16),
        "w2t": _t128(np.asarray(inputs["conv2_w"], np.float32).T).astype(bf16),
        "ftab": ftab.astype(bf16), "gtab": gtab.astype(bf16),
        "identt": ident, "identtb": ident.astype(bf16),
        "mtab": mtab.astype(bf16), "bnt": bnt, "bqkt": bqkt,
    }
    xb = x.astype(bf16)
    mloc = MODES // NCORES
    maps = []
    for c in range(NCORES):
        m = dict(com)
        m["xs"] = np.ascontiguousarray(xb[c * BPC:(c + 1) * BPC])
        m["wslab"] = np.ascontiguousarray(wslab[c * mloc:(c + 1) * mloc])
        maps.append(m)
    _CACHE["fp"] = fp
    _CACHE["in_maps"] = maps
    return maps


def kernel(**inputs):
    nc = _get_nc()
    in_maps = _host_inputs(inputs)
    trace = bool(int(os.environ.get("BK_TRACE", "0")))
    res = run_bass_kernel_spmd(nc, in_maps, core_ids=list(range(NCORES)),
                               trace=trace)
    if trace and res.exec_time_ns is not None:
        print(f"HW exec time: {res.exec_time_ns} ns")
        _CACHE["exec_time_ns"] = res.exec_time_ns
    out = np.concatenate([res.results[c]["out"] for c in range(NCORES)], 0)
    return out.astype(np.float32)
